# revision 5
# baseline (speedup 1.0000x reference)
"""Trainium2 Bass kernel for nn_CachedConvNet (6-layer dilated causal conv net,
gated residual blocks, ring-buffer cache).

Sharding: pure data parallel — batch element b runs on NeuronCore b (B=8,
n_cores=8). Inside each core, time is split into two halves packed into the
128 SBUF partitions (2 x 64 channels); the second half starts one tile early
(recompute warm-up) to break the sequential dependency between halves.

Per layer: the two 64->64 convs (filter, gate) are evaluated as block-diagonal
[128,128] matmuls (float32r, 1 cycle/row) with the K=3 dilated taps expressed
as column-shifted access patterns of one SBUF input tile. Tanh/sigmoid run as
full 128-partition ScalarE activations straight out of PSUM (conv bias folded
into the activation bias); the gated residual is two VectorE ops.
"""

from contextlib import ExitStack

import numpy as np

import concourse.bacc as bacc
import concourse.bass as bass
import concourse.tile as tile
from concourse import mybir
from concourse.bass_utils import run_bass_kernel_spmd

# ---- problem constants (hardcoded; must match the reference) ----
B = 8
L = 6
C = 64
K = 3
T = 65536
DILS = (1, 2, 4, 8, 16, 32)
BUFS = tuple((K - 1) * d for d in DILS)
OFFS = tuple(int(v) for v in np.concatenate([[0], np.cumsum(BUFS)]))
CTX_LEN = sum(BUFS)  # 126

N = 1024          # tile columns per tick
HALF = T // 2     # 32768
N_TICKS = HALF // N + 1   # 33 (tick 0 = B-half warm-up, A idle)
MM_N = 512        # one PSUM bank per sub-matmul

_F32 = mybir.dt.float32
_F32R = mybir.dt.float32r
_BF16 = mybir.dt.bfloat16
# stream/matmul dtype: "f32r" (full-ish precision) or "bf16" (faster DVE/LDW)
DT_MODE = "f32r"


def _enable_walrus_ldw_opt():
    """Let walrus elide redundant LDWEIGHTS between adjacent same-weight
    matmuls (our tap-outer/bank-inner order issues each lhsT twice in a row).
    The flag is hardcoded off in bass_utils; rewrite it on the walrus argv."""
    import concourse.bass_utils as _bu
    if getattr(_bu, "_ldw_opt_patched", False):
        return
    _orig = _bu.run_command

    def _patched(argv, **kw):
        argv = ["--enable-ldw-opt=true" if a == "--enable-ldw-opt=false" else a
                for a in argv]
        return _orig(argv, **kw)

    _bu.run_command = _patched
    _bu._ldw_opt_patched = True


def _build_program():
    _enable_walrus_ldw_opt()
    _DT = _F32R if DT_MODE == "f32r" else _BF16
    nc = bacc.Bacc("TRN2", target_bir_lowering=False, debug=False)

    x_d = nc.dram_tensor("x", [C, T], _DT, kind="ExternalInput").ap()
    ctx_d = nc.dram_tensor("ctx", [C, CTX_LEN], _DT, kind="ExternalInput").ap()
    wts_d = nc.dram_tensor("wts", [128, L * 2 * K, 128], _DT,
                           kind="ExternalInput").ap()
    bias_d = nc.dram_tensor("bias", [128, 2 * L], _F32, kind="ExternalInput").ap()
    y_d = nc.dram_tensor("y", [C, T], _F32, kind="ExternalOutput").ap()
    ctxo_d = nc.dram_tensor("ctx_out", [C, CTX_LEN], _DT,
                            kind="ExternalOutput").ap()

    with tile.TileContext(nc) as tc, ExitStack() as ctx:
        singles = ctx.enter_context(tc.tile_pool(name="singles", bufs=1))
        xpools = ctx.enter_context(tc.tile_pool(name="xtiles", bufs=2))
        fgpool = ctx.enter_context(tc.tile_pool(name="fg", bufs=2))
        opool = ctx.enter_context(tc.tile_pool(name="out", bufs=3))
        ppool = ctx.enter_context(tc.tile_pool(name="psum", bufs=2, space="PSUM"))

        wts_sb = singles.tile([128, L * 2 * K, 128], _DT)
        nc.sync.dma_start(out=wts_sb, in_=wts_d)
        bias_sb = singles.tile([128, 2 * L], _F32)
        nc.sync.dma_start(out=bias_sb, in_=bias_d)
        ctx_sb = singles.tile([64, CTX_LEN], _DT)
        nc.sync.dma_start(out=ctx_sb, in_=ctx_d)

        prev_x = [None] * L

        for k in range(N_TICKS):
            sa = (k - 1) * N          # A-half output start (valid k>=1)
            sb = HALF - N + k * N     # B-half output start

            cur_x = []
            h0 = 2 * DILS[0]
            x0 = xpools.tile([128, N + h0], _DT, tag="x0")
            a_lo = max(sa, 0)
            nc.sync.dma_start(out=x0[0:64, h0:h0 + N], in_=x_d[:, a_lo:a_lo + N])
            nc.sync.dma_start(out=x0[64:128, h0:h0 + N], in_=x_d[:, sb:sb + N])
            cur_x.append(x0)

            out_t = None
            for i in range(L):
                d = DILS[i]
                h = 2 * d
                xi = cur_x[i]
                if k == 0:
                    # warm-up tick: halo contents are don't-care, but must be
                    # finite and f32r-produced (codegen rejects f32r Memset)
                    nc.vector.tensor_copy(xi[0:64, 0:h], ctx_sb[:, 0:h])
                    nc.vector.tensor_copy(xi[64:128, 0:h], ctx_sb[:, 0:h])
                elif k == 1:
                    nc.vector.tensor_copy(xi[0:64, 0:h],
                                          ctx_sb[:, OFFS[i]:OFFS[i] + h])
                    nc.vector.tensor_copy(xi[64:128, 0:h],
                                          prev_x[i][64:128, N:N + h])
                else:
                    nc.vector.tensor_copy(xi[:, 0:h], prev_x[i][:, N:N + h])

                psum_f = ppool.tile([128, N], _F32, tag="pf")
                psum_g = ppool.tile([128, N], _F32, tag="pg")
                # tap-outer, bank-inner: adjacent matmuls share lhsT, and each
                # PSUM bank holds exactly one start/stop accumulation group.
                for w_idx, ps in ((0, psum_f), (1, psum_g)):
                    for j in range(K):
                        lhsT = wts_sb[:, (i * 2 + w_idx) * K + j, :]
                        for m0 in range(0, N, MM_N):
                            nc.tensor.matmul(
                                ps[:, m0:m0 + MM_N],
                                lhsT,
                                xi[:, j * d + m0: j * d + m0 + MM_N],
                                start=(j == 0),
                                stop=(j == K - 1),
                            )

                f_sb = fgpool.tile([128, N], _DT, tag="f")
                g_sb = fgpool.tile([128, N], _DT, tag="g")
                nc.scalar.activation(
                    f_sb, psum_f, mybir.ActivationFunctionType.Tanh,
                    bias=bias_sb[:, 2 * i:2 * i + 1], scale=1.0)
                nc.scalar.activation(
                    g_sb, psum_g, mybir.ActivationFunctionType.Sigmoid,
                    bias=bias_sb[:, 2 * i + 1:2 * i + 2], scale=1.0)

                prod = fgpool.tile([128, N], _DT, tag="p")
                nc.vector.tensor_mul(prod, f_sb, g_sb)
                if i + 1 < L:
                    hn = 2 * DILS[i + 1]
                    xn = xpools.tile([128, N + hn], _DT, tag=f"x{i + 1}")
                    nc.vector.tensor_add(xn[:, hn:hn + N], xi[:, h:h + N], prod)
                    cur_x.append(xn)
                else:
                    out_t = opool.tile([128, N], _F32, tag="o")
                    nc.vector.tensor_add(out_t, xi[:, h:h + N], prod)

            if k >= 1:
                nc.sync.dma_start(out=y_d[:, sa:sa + N], in_=out_t[0:64, :])
                nc.sync.dma_start(out=y_d[:, sb:sb + N], in_=out_t[64:128, :])

            if k == N_TICKS - 1:
                for i in range(L):
                    h = 2 * DILS[i]
                    nc.sync.dma_start(
                        out=ctxo_d[:, OFFS[i]:OFFS[i] + h],
                        in_=cur_x[i][64:128, N:N + h])

            prev_x = cur_x

    nc.compile()
    return nc


def _build_weight_blocks(Wf, Wg):
    """Host-side prep: [128, L*2*K, 128] block-diag lhsT tensors.

    wts[k, (i*2+w)*K + j, m] = W[i][m%64, k%64, j] on the two diagonal 64x64
    blocks (chunk A rows 0-63, chunk B rows 64-127), zero elsewhere.
    """
    wts = np.zeros((128, L * 2 * K, 128), np.float32)
    for i in range(L):
        for w_idx, W in enumerate((Wf, Wg)):
            for j in range(K):
                blk = np.ascontiguousarray(W[i, :, :, j].T)  # [c_in, c_out]
                s = (i * 2 + w_idx) * K + j
                wts[0:64, s, 0:64] = blk
                wts[64:128, s, 64:128] = blk
    return wts


def _build_bias(bf, bg):
    b = np.zeros((128, 2 * L), np.float32)
    for i in range(L):
        b[0:64, 2 * i] = bf[i]
        b[64:128, 2 * i] = bf[i]
        b[0:64, 2 * i + 1] = bg[i]
        b[64:128, 2 * i + 1] = bg[i]
    return b


_NC_CACHE = None


def kernel(x, ctx, Wf, bf, Wg, bg):
    global _NC_CACHE
    x = np.asarray(x, dtype=np.float32)
    ctx = np.asarray(ctx, dtype=np.float32)
    Wf = np.asarray(Wf, dtype=np.float32)
    bf = np.asarray(bf, dtype=np.float32)
    Wg = np.asarray(Wg, dtype=np.float32)
    bg = np.asarray(bg, dtype=np.float32)

    if _NC_CACHE is None:
        _NC_CACHE = _build_program()
    nc = _NC_CACHE

    wts = _build_weight_blocks(Wf, Wg)
    bias = _build_bias(bf, bg)
    if DT_MODE == "bf16":
        import ml_dtypes
        sdt = ml_dtypes.bfloat16
    else:
        sdt = np.float32
    in_maps = [
        {"x": np.ascontiguousarray(x[b]).astype(sdt),
         "ctx": np.ascontiguousarray(ctx[b]).astype(sdt),
         "wts": wts.astype(sdt),
         "bias": bias}
        for b in range(B)
    ]
    res = run_bass_kernel_spmd(nc, in_maps, core_ids=list(range(B)))
    y = np.stack([res.results[b]["y"] for b in range(B)]).astype(np.float32)
    ctx_out = np.stack(
        [res.results[b]["ctx_out"] for b in range(B)]).astype(np.float32)
    return y, ctx_out


# revision 6
# speedup vs baseline: 1.2740x; 1.2740x over previous
"""Trainium2 Bass kernel for nn_CachedConvNet (6-layer dilated causal conv net,
gated residual blocks, ring-buffer cache).

Sharding: pure data parallel — batch element b runs on NeuronCore b (B=8,
n_cores=8). Inside each core, time is split into two halves packed into the
128 SBUF partitions (2 x 64 channels); the second half starts one tile early
(recompute warm-up) to break the sequential dependency between halves.

Per layer: the two 64->64 convs (filter, gate) are evaluated as block-diagonal
[128,128] matmuls (float32r, 1 cycle/row) with the K=3 dilated taps expressed
as column-shifted access patterns of one SBUF input tile. Tanh/sigmoid run as
full 128-partition ScalarE activations straight out of PSUM (conv bias folded
into the activation bias); the gated residual is two VectorE ops.
"""

from contextlib import ExitStack

import numpy as np

import concourse.bacc as bacc
import concourse.bass as bass
import concourse.tile as tile
from concourse import mybir
from concourse.bass_utils import run_bass_kernel_spmd

# ---- problem constants (hardcoded; must match the reference) ----
B = 8
L = 6
C = 64
K = 3
T = 65536
DILS = (1, 2, 4, 8, 16, 32)
BUFS = tuple((K - 1) * d for d in DILS)
OFFS = tuple(int(v) for v in np.concatenate([[0], np.cumsum(BUFS)]))
CTX_LEN = sum(BUFS)  # 126

N = 1024          # tile columns per tick
HALF = T // 2     # 32768
N_TICKS = HALF // N + 1   # 33 (tick 0 = B-half warm-up, A idle)
MM_N = 512        # one PSUM bank per sub-matmul

_F32 = mybir.dt.float32
_F32R = mybir.dt.float32r
_BF16 = mybir.dt.bfloat16
# stream/matmul dtype: "f32r" (full-ish precision) or "bf16" (faster DVE/LDW)
DT_MODE = "f32r"


def _enable_walrus_ldw_opt():
    """Let walrus elide redundant LDWEIGHTS between adjacent same-weight
    matmuls (our tap-outer/bank-inner order issues each lhsT twice in a row).
    The flag is hardcoded off in bass_utils; rewrite it on the walrus argv."""
    import concourse.bass_utils as _bu
    if getattr(_bu, "_ldw_opt_patched", False):
        return
    _orig = _bu.run_command

    def _patched(argv, **kw):
        argv = ["--enable-ldw-opt=true" if a == "--enable-ldw-opt=false" else a
                for a in argv]
        return _orig(argv, **kw)

    _bu.run_command = _patched
    _bu._ldw_opt_patched = True


def _build_program():
    if DT_MODE == "f32r":
        # bf16 matmuls lower to explicit InstLdweights, which walrus's LDW
        # optimization rejects; only patch the flag for the f32r path.
        _enable_walrus_ldw_opt()
    _DT = _F32R if DT_MODE == "f32r" else _BF16
    nc = bacc.Bacc("TRN2", target_bir_lowering=False, debug=False)

    x_d = nc.dram_tensor("x", [C, T], _DT, kind="ExternalInput").ap()
    ctx_d = nc.dram_tensor("ctx", [C, CTX_LEN], _DT, kind="ExternalInput").ap()
    wts_d = nc.dram_tensor("wts", [128, L * 2 * K, 128], _DT,
                           kind="ExternalInput").ap()
    bias_d = nc.dram_tensor("bias", [128, 2 * L], _F32, kind="ExternalInput").ap()
    y_d = nc.dram_tensor("y", [C, T], _F32, kind="ExternalOutput").ap()
    ctxo_d = nc.dram_tensor("ctx_out", [C, CTX_LEN], _DT,
                            kind="ExternalOutput").ap()

    with tile.TileContext(nc) as tc, ExitStack() as ctx:
        singles = ctx.enter_context(tc.tile_pool(name="singles", bufs=1))
        xpools = ctx.enter_context(tc.tile_pool(name="xtiles", bufs=2))
        fgpool = ctx.enter_context(tc.tile_pool(name="fg", bufs=2))
        opool = ctx.enter_context(tc.tile_pool(name="out", bufs=3))
        ppool = ctx.enter_context(tc.tile_pool(name="psum", bufs=2, space="PSUM"))

        wts_sb = singles.tile([128, L * 2 * K, 128], _DT)
        nc.sync.dma_start(out=wts_sb, in_=wts_d)
        bias_sb = singles.tile([128, 2 * L], _F32)
        nc.sync.dma_start(out=bias_sb, in_=bias_d)
        ctx_sb = singles.tile([64, CTX_LEN], _DT)
        nc.sync.dma_start(out=ctx_sb, in_=ctx_d)

        prev_x = [None] * L

        for k in range(N_TICKS):
            sa = (k - 1) * N          # A-half output start (valid k>=1)
            sb = HALF - N + k * N     # B-half output start

            cur_x = []
            h0 = 2 * DILS[0]
            x0 = xpools.tile([128, N + h0], _DT, tag="x0")
            a_lo = max(sa, 0)
            nc.sync.dma_start(out=x0[0:64, h0:h0 + N], in_=x_d[:, a_lo:a_lo + N])
            nc.sync.dma_start(out=x0[64:128, h0:h0 + N], in_=x_d[:, sb:sb + N])
            cur_x.append(x0)

            out_t = None
            for i in range(L):
                d = DILS[i]
                h = 2 * d
                xi = cur_x[i]
                if k == 0:
                    # warm-up tick: halo contents are don't-care, but must be
                    # finite and f32r-produced (codegen rejects f32r Memset)
                    nc.vector.tensor_copy(xi[0:64, 0:h], ctx_sb[:, 0:h])
                    nc.vector.tensor_copy(xi[64:128, 0:h], ctx_sb[:, 0:h])
                elif k == 1:
                    nc.vector.tensor_copy(xi[0:64, 0:h],
                                          ctx_sb[:, OFFS[i]:OFFS[i] + h])
                    nc.vector.tensor_copy(xi[64:128, 0:h],
                                          prev_x[i][64:128, N:N + h])
                else:
                    nc.vector.tensor_copy(xi[:, 0:h], prev_x[i][:, N:N + h])

                psum_f = ppool.tile([128, N], _F32, tag="pf")
                psum_g = ppool.tile([128, N], _F32, tag="pg")
                # tap-outer, bank-inner: adjacent matmuls share lhsT, and each
                # PSUM bank holds exactly one start/stop accumulation group.
                for w_idx, ps in ((0, psum_f), (1, psum_g)):
                    for j in range(K):
                        lhsT = wts_sb[:, (i * 2 + w_idx) * K + j, :]
                        for m0 in range(0, N, MM_N):
                            nc.tensor.matmul(
                                ps[:, m0:m0 + MM_N],
                                lhsT,
                                xi[:, j * d + m0: j * d + m0 + MM_N],
                                start=(j == 0),
                                stop=(j == K - 1),
                            )

                f_sb = fgpool.tile([128, N], _DT, tag="f")
                g_sb = fgpool.tile([128, N], _DT, tag="g")
                nc.scalar.activation(
                    f_sb, psum_f, mybir.ActivationFunctionType.Tanh,
                    bias=bias_sb[:, 2 * i:2 * i + 1], scale=1.0)
                nc.scalar.activation(
                    g_sb, psum_g, mybir.ActivationFunctionType.Sigmoid,
                    bias=bias_sb[:, 2 * i + 1:2 * i + 2], scale=1.0)

                prod = fgpool.tile([128, N], _DT, tag="p")
                nc.vector.tensor_mul(prod, f_sb, g_sb)
                if i + 1 < L:
                    hn = 2 * DILS[i + 1]
                    xn = xpools.tile([128, N + hn], _DT, tag=f"x{i + 1}")
                    nc.vector.tensor_add(xn[:, hn:hn + N], xi[:, h:h + N], prod)
                    cur_x.append(xn)
                else:
                    out_t = opool.tile([128, N], _F32, tag="o")
                    nc.vector.tensor_add(out_t, xi[:, h:h + N], prod)

            if k >= 1:
                nc.sync.dma_start(out=y_d[:, sa:sa + N], in_=out_t[0:64, :])
                nc.sync.dma_start(out=y_d[:, sb:sb + N], in_=out_t[64:128, :])

            if k == N_TICKS - 1:
                for i in range(L):
                    h = 2 * DILS[i]
                    nc.sync.dma_start(
                        out=ctxo_d[:, OFFS[i]:OFFS[i] + h],
                        in_=cur_x[i][64:128, N:N + h])

            prev_x = cur_x

    nc.compile()
    return nc


def _build_weight_blocks(Wf, Wg):
    """Host-side prep: [128, L*2*K, 128] block-diag lhsT tensors.

    wts[k, (i*2+w)*K + j, m] = W[i][m%64, k%64, j] on the two diagonal 64x64
    blocks (chunk A rows 0-63, chunk B rows 64-127), zero elsewhere.
    """
    wts = np.zeros((128, L * 2 * K, 128), np.float32)
    for i in range(L):
        for w_idx, W in enumerate((Wf, Wg)):
            for j in range(K):
                blk = np.ascontiguousarray(W[i, :, :, j].T)  # [c_in, c_out]
                s = (i * 2 + w_idx) * K + j
                wts[0:64, s, 0:64] = blk
                wts[64:128, s, 64:128] = blk
    return wts


def _build_bias(bf, bg):
    b = np.zeros((128, 2 * L), np.float32)
    for i in range(L):
        b[0:64, 2 * i] = bf[i]
        b[64:128, 2 * i] = bf[i]
        b[0:64, 2 * i + 1] = bg[i]
        b[64:128, 2 * i + 1] = bg[i]
    return b


_NC_CACHE = None


def kernel(x, ctx, Wf, bf, Wg, bg):
    global _NC_CACHE
    x = np.asarray(x, dtype=np.float32)
    ctx = np.asarray(ctx, dtype=np.float32)
    Wf = np.asarray(Wf, dtype=np.float32)
    bf = np.asarray(bf, dtype=np.float32)
    Wg = np.asarray(Wg, dtype=np.float32)
    bg = np.asarray(bg, dtype=np.float32)

    if _NC_CACHE is None:
        _NC_CACHE = _build_program()
    nc = _NC_CACHE

    wts = _build_weight_blocks(Wf, Wg)
    bias = _build_bias(bf, bg)
    if DT_MODE == "bf16":
        import ml_dtypes
        sdt = ml_dtypes.bfloat16
    else:
        sdt = np.float32
    in_maps = [
        {"x": np.ascontiguousarray(x[b]).astype(sdt),
         "ctx": np.ascontiguousarray(ctx[b]).astype(sdt),
         "wts": wts.astype(sdt),
         "bias": bias}
        for b in range(B)
    ]
    res = run_bass_kernel_spmd(nc, in_maps, core_ids=list(range(B)))
    y = np.stack([res.results[b]["y"] for b in range(B)]).astype(np.float32)
    ctx_out = np.stack(
        [res.results[b]["ctx_out"] for b in range(B)]).astype(np.float32)
    return y, ctx_out


# revision 7
# speedup vs baseline: 2.9668x; 2.3287x over previous
"""Trainium2 Bass kernel for nn_CachedConvNet (6-layer dilated causal conv net,
gated residual blocks, ring-buffer cache).

Sharding: pure data parallel — batch element b runs on NeuronCore b (B=8,
n_cores=8). Inside each core, time is split into two halves packed into the
128 SBUF partitions (2 x 64 channels); the second half starts one tile early
(recompute warm-up) to break the sequential dependency between halves.

Per layer: the two 64->64 convs (filter, gate) are evaluated as block-diagonal
[128,128] matmuls (float32r, 1 cycle/row) with the K=3 dilated taps expressed
as column-shifted access patterns of one SBUF input tile. Tanh/sigmoid run as
full 128-partition ScalarE activations straight out of PSUM (conv bias folded
into the activation bias); the gated residual is two VectorE ops.
"""

from contextlib import ExitStack

import numpy as np

import concourse.bacc as bacc
import concourse.bass as bass
import concourse.tile as tile
from concourse import mybir
from concourse.bass_utils import run_bass_kernel_spmd

# ---- problem constants (hardcoded; must match the reference) ----
B = 8
L = 6
C = 64
K = 3
T = 65536
DILS = (1, 2, 4, 8, 16, 32)
BUFS = tuple((K - 1) * d for d in DILS)
OFFS = tuple(int(v) for v in np.concatenate([[0], np.cumsum(BUFS)]))
CTX_LEN = sum(BUFS)  # 126

N = 1024          # tile columns per tick
HALF = T // 2     # 32768
N_TICKS = HALF // N + 1   # 33 (tick 0 = B-half warm-up, A idle)
MM_N = 512        # one PSUM bank per sub-matmul

_F32 = mybir.dt.float32
_F32R = mybir.dt.float32r
_BF16 = mybir.dt.bfloat16
# stream/matmul dtype: "f32r" (full-ish precision) or "bf16" (faster DVE/LDW)
DT_MODE = "f32r"


def _enable_walrus_ldw_opt():
    """Let walrus elide redundant LDWEIGHTS between adjacent same-weight
    matmuls (our tap-outer/bank-inner order issues each lhsT twice in a row).
    The flag is hardcoded off in bass_utils; rewrite it on the walrus argv."""
    import concourse.bass_utils as _bu
    if getattr(_bu, "_ldw_opt_patched", False):
        return
    _orig = _bu.run_command

    def _patched(argv, **kw):
        argv = ["--enable-ldw-opt=true" if a == "--enable-ldw-opt=false" else a
                for a in argv]
        return _orig(argv, **kw)

    _bu.run_command = _patched
    _bu._ldw_opt_patched = True


def _build_program():
    if DT_MODE == "f32r":
        # bf16 matmuls lower to explicit InstLdweights, which walrus's LDW
        # optimization rejects; only patch the flag for the f32r path.
        _enable_walrus_ldw_opt()
    _DT = _F32R if DT_MODE == "f32r" else _BF16
    nc = bacc.Bacc("TRN2", target_bir_lowering=False, debug=False)

    x_d = nc.dram_tensor("x", [C, T], _DT, kind="ExternalInput").ap()
    ctx_d = nc.dram_tensor("ctx", [C, CTX_LEN], _DT, kind="ExternalInput").ap()
    wts_d = nc.dram_tensor("wts", [128, L * 2 * K, 128], _DT,
                           kind="ExternalInput").ap()
    bias_d = nc.dram_tensor("bias", [128, 2 * L], _F32, kind="ExternalInput").ap()
    y_d = nc.dram_tensor("y", [C, T], _F32, kind="ExternalOutput").ap()
    ctxo_d = nc.dram_tensor("ctx_out", [C, CTX_LEN], _DT,
                            kind="ExternalOutput").ap()

    with tile.TileContext(nc) as tc, ExitStack() as ctx:
        singles = ctx.enter_context(tc.tile_pool(name="singles", bufs=1))
        xpools = ctx.enter_context(tc.tile_pool(name="xtiles", bufs=2))
        fgpool = ctx.enter_context(tc.tile_pool(name="fg", bufs=2))
        opool = ctx.enter_context(tc.tile_pool(name="out", bufs=3))
        ppool = ctx.enter_context(tc.tile_pool(name="psum", bufs=2, space="PSUM"))

        wts_sb = singles.tile([128, L * 2 * K, 128], _DT)
        nc.sync.dma_start(out=wts_sb, in_=wts_d)
        bias_sb = singles.tile([128, 2 * L], _F32)
        nc.sync.dma_start(out=bias_sb, in_=bias_d)
        ctx_sb = singles.tile([64, CTX_LEN], _DT)
        nc.sync.dma_start(out=ctx_sb, in_=ctx_d)

        # Skewed software pipeline: at super-tick s, layer i processes data
        # tick t = s - i. The six layer-blocks emitted per super-tick are
        # mutually independent (layer i's input for tick t was produced a
        # full super-tick earlier), so no engine stalls on the intra-tick
        # chain mm -> ACT -> mul -> add and the PE stays dense (HAM-warm).
        xt = {}  # (layer, tick) -> input tile

        for s in range(N_TICKS + L - 1):
            for i in range(L):
                t = s - i
                if not (0 <= t < N_TICKS):
                    continue
                d = DILS[i]
                h = 2 * d

                if i == 0:
                    sa = (t - 1) * N          # A-half output start (t>=1)
                    sb = HALF - N + t * N     # B-half output start
                    x0 = xpools.tile([128, N + h], _DT, tag="x0")
                    a_lo = max(sa, 0)
                    nc.sync.dma_start(out=x0[0:64, h:h + N],
                                      in_=x_d[:, a_lo:a_lo + N])
                    nc.sync.dma_start(out=x0[64:128, h:h + N],
                                      in_=x_d[:, sb:sb + N])
                    xt[(0, t)] = x0

                xi = xt[(i, t)]
                if t == 0:
                    # warm-up tick: halo contents are don't-care, but must be
                    # finite and _DT-produced (codegen rejects f32r Memset)
                    nc.vector.tensor_copy(xi[0:64, 0:h], ctx_sb[:, 0:h])
                    nc.vector.tensor_copy(xi[64:128, 0:h], ctx_sb[:, 0:h])
                elif t == 1:
                    nc.vector.tensor_copy(xi[0:64, 0:h],
                                          ctx_sb[:, OFFS[i]:OFFS[i] + h])
                    nc.vector.tensor_copy(xi[64:128, 0:h],
                                          xt[(i, 0)][64:128, N:N + h])
                else:
                    nc.vector.tensor_copy(xi[:, 0:h],
                                          xt[(i, t - 1)][:, N:N + h])

                psum_f = ppool.tile([128, N], _F32, tag="pf")
                psum_g = ppool.tile([128, N], _F32, tag="pg")
                # tap-outer, bank-inner: adjacent matmuls share lhsT, and each
                # PSUM bank holds exactly one start/stop accumulation group.
                for w_idx, ps in ((0, psum_f), (1, psum_g)):
                    for j in range(K):
                        lhsT = wts_sb[:, (i * 2 + w_idx) * K + j, :]
                        for m0 in range(0, N, MM_N):
                            nc.tensor.matmul(
                                ps[:, m0:m0 + MM_N],
                                lhsT,
                                xi[:, j * d + m0: j * d + m0 + MM_N],
                                start=(j == 0),
                                stop=(j == K - 1),
                            )

                f_sb = fgpool.tile([128, N], _DT, tag="f")
                g_sb = fgpool.tile([128, N], _DT, tag="g")
                nc.scalar.activation(
                    f_sb, psum_f, mybir.ActivationFunctionType.Tanh,
                    bias=bias_sb[:, 2 * i:2 * i + 1], scale=1.0)
                nc.scalar.activation(
                    g_sb, psum_g, mybir.ActivationFunctionType.Sigmoid,
                    bias=bias_sb[:, 2 * i + 1:2 * i + 2], scale=1.0)

                prod = fgpool.tile([128, N], _DT, tag="p")
                nc.vector.tensor_mul(prod, f_sb, g_sb)
                if i + 1 < L:
                    hn = 2 * DILS[i + 1]
                    xn = xpools.tile([128, N + hn], _DT, tag=f"x{i + 1}")
                    nc.vector.tensor_add(xn[:, hn:hn + N], xi[:, h:h + N], prod)
                    xt[(i + 1, t)] = xn
                else:
                    out_t = opool.tile([128, N], _F32, tag="o")
                    nc.vector.tensor_add(out_t, xi[:, h:h + N], prod)
                    if t >= 1:
                        sa = (t - 1) * N
                        sb = HALF - N + t * N
                        nc.sync.dma_start(out=y_d[:, sa:sa + N],
                                          in_=out_t[0:64, :])
                        nc.sync.dma_start(out=y_d[:, sb:sb + N],
                                          in_=out_t[64:128, :])

                if t == N_TICKS - 1:
                    nc.sync.dma_start(
                        out=ctxo_d[:, OFFS[i]:OFFS[i] + h],
                        in_=xi[64:128, N:N + h])

                xt.pop((i, t - 1), None)

    nc.compile()
    return nc


def _build_weight_blocks(Wf, Wg):
    """Host-side prep: [128, L*2*K, 128] block-diag lhsT tensors.

    wts[k, (i*2+w)*K + j, m] = W[i][m%64, k%64, j] on the two diagonal 64x64
    blocks (chunk A rows 0-63, chunk B rows 64-127), zero elsewhere.
    """
    wts = np.zeros((128, L * 2 * K, 128), np.float32)
    for i in range(L):
        for w_idx, W in enumerate((Wf, Wg)):
            for j in range(K):
                blk = np.ascontiguousarray(W[i, :, :, j].T)  # [c_in, c_out]
                s = (i * 2 + w_idx) * K + j
                wts[0:64, s, 0:64] = blk
                wts[64:128, s, 64:128] = blk
    return wts


def _build_bias(bf, bg):
    b = np.zeros((128, 2 * L), np.float32)
    for i in range(L):
        b[0:64, 2 * i] = bf[i]
        b[64:128, 2 * i] = bf[i]
        b[0:64, 2 * i + 1] = bg[i]
        b[64:128, 2 * i + 1] = bg[i]
    return b


_NC_CACHE = None


def kernel(x, ctx, Wf, bf, Wg, bg):
    global _NC_CACHE
    x = np.asarray(x, dtype=np.float32)
    ctx = np.asarray(ctx, dtype=np.float32)
    Wf = np.asarray(Wf, dtype=np.float32)
    bf = np.asarray(bf, dtype=np.float32)
    Wg = np.asarray(Wg, dtype=np.float32)
    bg = np.asarray(bg, dtype=np.float32)

    if _NC_CACHE is None:
        _NC_CACHE = _build_program()
    nc = _NC_CACHE

    wts = _build_weight_blocks(Wf, Wg)
    bias = _build_bias(bf, bg)
    if DT_MODE == "bf16":
        import ml_dtypes
        sdt = ml_dtypes.bfloat16
    else:
        sdt = np.float32
    in_maps = [
        {"x": np.ascontiguousarray(x[b]).astype(sdt),
         "ctx": np.ascontiguousarray(ctx[b]).astype(sdt),
         "wts": wts.astype(sdt),
         "bias": bias}
        for b in range(B)
    ]
    res = run_bass_kernel_spmd(nc, in_maps, core_ids=list(range(B)))
    y = np.stack([res.results[b]["y"] for b in range(B)]).astype(np.float32)
    ctx_out = np.stack(
        [res.results[b]["ctx_out"] for b in range(B)]).astype(np.float32)
    return y, ctx_out
